# revision 1
# baseline (speedup 1.0000x reference)
"""Trainium2 Bass kernel: single-head causal attention (B=2, S=4096, E=1024, H=128).

Sharding: 8 cores = 2 batches x 4 query-quarters. Core (b, c) computes global
query tiles {c + 4m : m=0..7} (128 rows each). To keep the SPMD program uniform
across cores despite causality, each core's K/V blocks are host-permuted so that
query tile m's causal diagonal always lands at local kv block 4m+3, with
full-attention blocks packed into slots [0, 4m+3) and zero pads elsewhere.
Device-side masking is then core-independent: a fixed tril mask on the last
block of each tile's span (plus an explicit per-core mask for tile 0).

Matmul dtypes: float32r (tf32-like fast fp32 path) for projections and scores;
bf16 for the attention-weights path (exp output, transposes, attn @ v).
"""
import sys
import numpy as np

if "/opt/trn_rl_repo" not in sys.path:
    sys.path.insert(0, "/opt/trn_rl_repo")

import ml_dtypes
from contextlib import ExitStack

import concourse.bass as bass
import concourse.tile as tile
from concourse import bacc, mybir
from concourse.bass_utils import run_bass_kernel_spmd

P = 128
S = 4096
E = 1024
NE = E // P          # 8 contraction tiles
NQ = 8               # q tiles per core
NKV = S // P         # 32 kv blocks
QROWS = NQ * P       # 1024 q rows per core
F32 = mybir.dt.float32
F32R = mybir.dt.float32r
BF16 = mybir.dt.bfloat16

_CACHE = {}


def _build():
    nc = bacc.Bacc("TRN2", target_bir_lowering=False, debug=False, num_devices=8)
    xth = nc.dram_tensor("xth", [E, S], BF16, kind="ExternalInput")
    xtl = nc.dram_tensor("xtl", [E, S], BF16, kind="ExternalInput")
    wqh = nc.dram_tensor("wqh", [E, P], BF16, kind="ExternalInput")
    wql = nc.dram_tensor("wql", [E, P], BF16, kind="ExternalInput")
    wkh = nc.dram_tensor("wkh", [E, P], BF16, kind="ExternalInput")
    wkl = nc.dram_tensor("wkl", [E, P], BF16, kind="ExternalInput")
    wvh = nc.dram_tensor("wvh", [E, P], BF16, kind="ExternalInput")
    cbq = nc.dram_tensor("cbq", [1, P], BF16, kind="ExternalInput")
    ones = nc.dram_tensor("ones", [1, 512], BF16, kind="ExternalInput")
    bvb = nc.dram_tensor("bvb", [P, P], F32, kind="ExternalInput")
    mask0 = nc.dram_tensor("mask0", [P, 512], F32, kind="ExternalInput")
    tril = nc.dram_tensor("tril", [P, P], F32, kind="ExternalInput")
    idn = nc.dram_tensor("idn", [P, P], BF16, kind="ExternalInput")
    out = nc.dram_tensor("out", [QROWS, P], F32, kind="ExternalOutput")

    with tile.TileContext(nc) as tc, ExitStack() as ctx:
        const = ctx.enter_context(tc.tile_pool(name="const", bufs=1))
        t_cbq = const.tile([1, P], BF16, tag="cbq")
        nc.sync.dma_start(t_cbq[:], cbq.ap()[:, :])
        t_ones = const.tile([1, 512], BF16, tag="ones")
        nc.sync.dma_start(t_ones[:], ones.ap()[:, :])
        t_bvb = const.tile([P, P], F32, tag="bvb")
        nc.sync.dma_start(t_bvb[:], bvb.ap()[:, :])
        t_mask0 = const.tile([P, 512], F32, tag="mask0")
        nc.sync.dma_start(t_mask0[:], mask0.ap()[:, :])
        t_tril = const.tile([P, P], F32, tag="tril")
        nc.sync.dma_start(t_tril[:], tril.ap()[:, :])
        t_idn = const.tile([P, P], BF16, tag="idn")
        nc.sync.dma_start(t_idn[:], idn.ap()[:, :])

        # persistent phase-1 outputs
        proj = ctx.enter_context(tc.tile_pool(name="proj", bufs=1))
        kTh = proj.tile([P, S], BF16, tag="kTh")        # [h, kv]
        kTl = proj.tile([P, S], BF16, tag="kTl")
        qTh = proj.tile([P, QROWS], BF16, tag="qTh")    # [h, q]
        qTl = proj.tile([P, QROWS], BF16, tag="qTl")
        vTs = proj.tile([P, S], BF16, tag="vT")         # [h, kv]
        vs = proj.tile([P, S], BF16, tag="v")           # 32 blocks of [kv128, h128]

        # ---------------- phase 1: projections ----------------
        with ExitStack() as p1:
            xp = p1.enter_context(tc.tile_pool(name="xt", bufs=1))
            wp = p1.enter_context(tc.tile_pool(name="w", bufs=1))
            pk = p1.enter_context(tc.tile_pool(name="pk", bufs=4, space="PSUM"))
            pv = p1.enter_context(tc.tile_pool(name="pv", bufs=2, space="PSUM"))
            pq = p1.enter_context(tc.tile_pool(name="pq", bufs=1, space="PSUM"))
            ptv = p1.enter_context(tc.tile_pool(name="ptv", bufs=1, space="PSUM"))

            xhs, xls = [], []
            wd = {n: [] for n in ("qh", "ql", "kh", "kl", "vh")}
            for e in range(NE):
                th = xp.tile([P, S], BF16, tag=f"xth{e}")
                nc.sync.dma_start(th[:], xth.ap()[e * P:(e + 1) * P, :])
                xhs.append(th)
                tl = xp.tile([P, S], BF16, tag=f"xtl{e}")
                nc.sync.dma_start(tl[:], xtl.ap()[e * P:(e + 1) * P, :])
                xls.append(tl)
                for nm, dram in (("qh", wqh), ("ql", wql), ("kh", wkh),
                                 ("kl", wkl), ("vh", wvh)):
                    w = wp.tile([P, P], BF16, tag=f"w{nm}{e}")
                    nc.sync.dma_start(w[:], dram.ap()[e * P:(e + 1) * P, :])
                    wd[nm].append(w)

            for half in range(2):  # kT: e-outer over 4-chunk halves (DMA overlap)
                pses = [pk.tile([P, 512], F32, name=f"pkt{ni}", tag="pk") for ni in range(4)]
                for e in range(NE):
                    for ni in range(4):
                        sl = bass.ts(half * 4 + ni, 512)
                        nc.tensor.matmul(pses[ni][:], wd["kh"][e][:], xhs[e][:, sl],
                                         start=(e == 0), stop=False)
                        nc.tensor.matmul(pses[ni][:], wd["kh"][e][:], xls[e][:, sl],
                                         start=False, stop=False)
                        nc.tensor.matmul(pses[ni][:], wd["kl"][e][:], xhs[e][:, sl],
                                         start=False, stop=(e == NE - 1))
                for ni in range(4):
                    sl = bass.ts(half * 4 + ni, 512)
                    nc.scalar.activation(kTh[:, sl], pses[ni][:],
                                         mybir.ActivationFunctionType.Copy)
                    nc.vector.tensor_sub(kTl[:, sl], pses[ni][:], kTh[:, sl])
            for n in range(NE):  # vT over kv chunks of 512
                sl = bass.ts(n, 512)
                ps2 = pv.tile([P, 512], F32, tag="pv")
                for e in range(NE):
                    nc.tensor.matmul(ps2[:], wd["vh"][e][:], xhs[e][:, sl],
                                     start=(e == 0), stop=(e == NE - 1))
                nc.scalar.activation(vTs[:, sl], ps2[:], mybir.ActivationFunctionType.Copy)

            for h in range(2):  # qT: gather diag slots {4m+3}
                ps = pq.tile([P, 512], F32, tag="pq")
                for e in range(NE):
                    dh = xhs[e][:].rearrange("p (g f b) -> p g f b", f=4, b=P)[:, :, 3, :]
                    dl = xls[e][:].rearrange("p (g f b) -> p g f b", f=4, b=P)[:, :, 3, :]
                    nc.tensor.matmul(ps[:], wd["qh"][e][:], dh[:, h * 4:(h + 1) * 4, :],
                                     start=(e == 0), stop=False)
                    nc.tensor.matmul(ps[:], wd["qh"][e][:], dl[:, h * 4:(h + 1) * 4, :],
                                     start=False, stop=False)
                    nc.tensor.matmul(ps[:], wd["ql"][e][:], dh[:, h * 4:(h + 1) * 4, :],
                                     start=False, stop=False)
                nc.tensor.matmul(ps[:], t_cbq[:], t_ones[:], start=False, stop=True)
                hs = bass.ts(h, 512)
                nc.scalar.activation(qTh[:, hs], ps[:], mybir.ActivationFunctionType.Copy)
                nc.vector.tensor_sub(qTl[:, hs], ps[:], qTh[:, hs])

            for j in range(NKV):  # v blocks: transpose vT
                pt_ = ptv.tile([P, P], BF16, tag="ptv")
                nc.tensor.transpose(pt_[:], vTs[:, bass.ts(j, P)], t_idn[:])
                nc.vector.tensor_copy(vs[:, bass.ts(j, P)], pt_[:])

        # ---------------- phase 2: attention ----------------
        sb = ctx.enter_context(tc.tile_pool(name="sbuf2", bufs=2))
        smalls = ctx.enter_context(tc.tile_pool(name="smalls", bufs=4))
        osb = ctx.enter_context(tc.tile_pool(name="osb", bufs=2))
        pscore = ctx.enter_context(tc.tile_pool(name="ps", bufs=2, space="PSUM"))
        pt = ctx.enter_context(tc.tile_pool(name="pt", bufs=2, space="PSUM"))
        po = ctx.enter_context(tc.tile_pool(name="po", bufs=2, space="PSUM"))

        for m in range(NQ):
            L = 512 * (m + 1)
            s_sb = sb.tile([P, S], F32, tag="s")
            attn = sb.tile([P, S], BF16, tag="attn")
            attnT = sb.tile([P, S], BF16, tag="attnT")
            lqh = qTh[:, bass.ts(m, P)]
            lql = qTl[:, bass.ts(m, P)]
            for n in range(m + 1):
                ns = bass.ts(n, 512)
                ps = pscore.tile([P, 512], F32, tag="ps")
                nc.tensor.matmul(ps[:], lqh, kTh[:, ns], start=True, stop=False)
                nc.tensor.matmul(ps[:], lqh, kTl[:, ns], start=False, stop=False)
                nc.tensor.matmul(ps[:], lql, kTh[:, ns], start=False, stop=True)
                if m == 0:
                    nc.vector.tensor_add(s_sb[:, 0:512], ps[:], t_mask0[:])
                elif n < m:
                    nc.scalar.activation(s_sb[:, bass.ts(n, 512)], ps[:],
                                         mybir.ActivationFunctionType.Copy)
                else:
                    nc.scalar.activation(s_sb[:, n * 512:n * 512 + 384], ps[:, 0:384],
                                         mybir.ActivationFunctionType.Copy)
                    nc.vector.tensor_add(s_sb[:, L - P:L], ps[:, 384:512], t_tril[:])
            mx = smalls.tile([P, 1], F32, tag="mx")
            nc.vector.reduce_max(mx[:], s_sb[:, :L], axis=mybir.AxisListType.X)
            ngm = smalls.tile([P, 1], F32, tag="ngm")
            nc.vector.tensor_scalar_mul(ngm[:], mx[:], -1.0)
            rs = smalls.tile([P, 1], F32, tag="rs")
            nc.scalar.activation(attn[:, :L], s_sb[:, :L],
                                 mybir.ActivationFunctionType.Exp,
                                 bias=ngm[:], scale=1.0, accum_out=rs[:])
            rcp = smalls.tile([P, 1], F32, tag="rcp")
            nc.vector.reciprocal(rcp[:], rs[:])
            nb = L // P
            for j in range(nb):
                ptj = pt.tile([P, P], BF16, tag="pt")
                nc.tensor.transpose(ptj[:], attn[:, bass.ts(j, P)], t_idn[:])
                eng = nc.vector if (j % 2 == 0) else nc.scalar
                if eng is nc.vector:
                    nc.vector.tensor_copy(attnT[:, bass.ts(j, P)], ptj[:])
                else:
                    nc.scalar.activation(attnT[:, bass.ts(j, P)], ptj[:],
                                         mybir.ActivationFunctionType.Copy)
            pom = po.tile([P, P], F32, tag="po")
            for j in range(nb):
                nc.tensor.matmul(pom[:], attnT[:, bass.ts(j, P)], vs[:, bass.ts(j, P)],
                                 start=(j == 0), stop=(j == nb - 1))
            ot = osb.tile([P, P], F32, tag="ot")
            nc.vector.scalar_tensor_tensor(ot[:], pom[:], rcp[:], t_bvb[:],
                                           op0=mybir.AluOpType.mult,
                                           op1=mybir.AluOpType.add)
            nc.sync.dma_start(out.ap()[bass.ts(m, P), :], ot[:])

    nc.compile()
    return nc


def _host_prep(input, Wq, bq, Wk, bk, Wv, bv):
    c = np.float32(np.sqrt(np.float32(P)))
    bf = ml_dtypes.bfloat16

    def split(a):
        a = np.asarray(a, np.float32)
        hi = a.astype(bf)
        lo = (a - hi.astype(np.float32)).astype(bf)
        return hi, lo

    wq_h, wq_l = split(np.asarray(Wq, np.float32) * c)
    wk_h, wk_l = split(Wk)
    wv_h = np.asarray(Wv, np.float32).astype(bf)
    cbq = (np.asarray(bq, np.float32) * c).astype(bf).reshape(1, P)
    ones = np.ones((1, 512), bf)
    bvb = np.broadcast_to(bv.astype(np.float32), (P, P)).copy()
    neg = np.float32(-1e30)
    trilm = np.where(np.tril(np.ones((P, P), bool)), np.float32(0), neg)
    idn = np.eye(P, dtype=ml_dtypes.bfloat16)

    in_maps = []
    metas = []
    for core in range(8):
        b, cq = divmod(core, 4)
        assign = {}
        for m in range(NQ):
            assign[4 * m + 3] = cq + 4 * m
            if m == 0:
                for g in range(cq):
                    assign[g] = g
            else:
                for t, g in enumerate(range(cq + 4 * m - 3, cq + 4 * m)):
                    assign[4 * m + t] = g
        X = np.asarray(input[b], np.float32)
        XT = np.zeros((E, S), np.float32)
        for slot, g in assign.items():
            XT[:, slot * P:(slot + 1) * P] = X[g * P:(g + 1) * P, :].T
        m0 = np.full((P, 512), neg, np.float32)
        m0[:, :cq * P] = 0.0
        m0[:, 384:512] = trilm
        xh, xl = split(XT)
        in_maps.append({
            "xth": xh, "xtl": xl, "wqh": wq_h, "wql": wq_l,
            "wkh": wk_h, "wkl": wk_l, "wvh": wv_h, "cbq": cbq, "ones": ones,
            "bvb": bvb, "mask0": m0, "tril": trilm, "idn": idn,
        })
        metas.append((b, cq))
    return in_maps, metas


def kernel(input, Wq, bq, Wk, bk, Wv, bv, _trace=False):
    if "nc" not in _CACHE:
        _CACHE["nc"] = _build()
    nc = _CACHE["nc"]
    in_maps, metas = _host_prep(np.asarray(input), np.asarray(Wq), np.asarray(bq),
                                np.asarray(Wk), np.asarray(bk),
                                np.asarray(Wv), np.asarray(bv))
    try:
        res = run_bass_kernel_spmd(nc, in_maps, list(range(8)), trace=_trace)
    except ModuleNotFoundError:
        res = run_bass_kernel_spmd(nc, in_maps, list(range(8)), trace=False)
    _CACHE["last_result"] = res
    B = 2
    full = np.zeros((B, S, P), np.float32)
    for core, (b, cq) in enumerate(metas):
        o = res.results[core]["out"]
        for m in range(NQ):
            g = cq + 4 * m
            full[b, g * P:(g + 1) * P, :] = o[m * P:(m + 1) * P, :]
    return full

